# revision 18
# baseline (speedup 1.0000x reference)
# kernel.py — self-contained Trainium2 Bass kernel for nn_BTDG_31774168055963 (moe_routing)
#
# Reference computation:
#   branch1: x1 = BN(S1[s]); pe1 = einsum('be,bef->bf', x1, (P1[p] @ G1.reshape(rd,ed*ed)).reshape(-1,ed,ed))
#            pe1 = BN(pe1); pred1 = pe1 @ O1.T
#   branch2: x2 = BN(S2[s]); m1 = x2*T_S[times]; per-coarse-time Tucker core G2[c]
#            pe2 = sum_c [c==fine2coarse[times]] einsum(m1, (P2[p] @ G2[c].reshape(rd,ed*ed)).reshape(-1,ed,ed))
#            pe2 = BN(pe2 * T_O[times]); pred2 = pe2 @ O2.T
#   out = sigmoid(pred1 + pred2)
#
# Strategy (8 NeuronCores):
#   - shard Tucker rank rd=200 -> 25 per core; host sorts samples by coarse bucket
#   - Tucker via outer-product moving operand Z[(e),b] = pp[r,b]*x[e,b]; pp replicated
#     across partitions with gpsimd.partition_broadcast (no DMA broadcast traffic)
#   - branch 1 (resident G1) first for a fast start; G2 streams/prefetches during it;
#     each branch split into two r-halves, each half's partial pe AllReduced (bf16)
#     while the next slab computes -> only the last AR is exposed
#   - PSUM evictions + BN applies on the scalar engine; half-sums via DMA accumulate;
#     vector keeps z-multiplies and BN statistics
#   - logits matmul sharded column-wise over E=20000 -> 2500/core; sigmoid bf16 out

import numpy as np
import ml_dtypes

BF16 = ml_dtypes.bfloat16

B, E, R2, T, C, ED, RD = 2048, 20000, 500, 365, 12, 200, 200
NCORES = 8
RS = RD // NCORES       # 25 r's per core
ES = E // NCORES        # 2500 vocab per core
BN_EPS = 1e-5
RHALVES = [(0, 13), (13, 25)]

_cache = {}


def _build(pieces):
    """Build + compile the per-core bass kernel. `pieces` is a tuple of
    (coarse_id, col_off, col_len) for branch-2 bucket matmuls."""
    import concourse.bass as bass
    import concourse.mybir as mybir
    import concourse.tile as tile
    from concourse import bacc

    f32 = mybir.dt.float32
    bf16 = mybir.dt.bfloat16
    AF = mybir.ActivationFunctionType

    nc = bacc.Bacc("TRN2", target_bir_lowering=False, debug=False, num_devices=NCORES)

    # ---------------- I/O ----------------
    x1_in = nc.dram_tensor("x1_in", [ED, B], bf16, kind="ExternalInput")   # S1[s_p].T
    x2_in = nc.dram_tensor("x2_in", [ED, B], bf16, kind="ExternalInput")   # S2[s_p].T
    ts_in = nc.dram_tensor("ts_in", [ED, B], bf16, kind="ExternalInput")   # T_S[times_p].T
    to_in = nc.dram_tensor("to_in", [ED, B], bf16, kind="ExternalInput")   # T_O[times_p].T
    g1_in = nc.dram_tensor("g1_in", [100, RS, 2, ED], bf16, kind="ExternalInput")
    g2_in = nc.dram_tensor("g2_in", [RS, 2, 100, C, ED], bf16, kind="ExternalInput")
    pb1_in = nc.dram_tensor("pb1_in", [RS, B], bf16, kind="ExternalInput")  # P1[p_p].T r-slice
    pb2_in = nc.dram_tensor("pb2_in", [RS, B], bf16, kind="ExternalInput")
    # O chunks: feat layout {0:128, 128:200} x {O1, O2}
    oc0_in = nc.dram_tensor("oc0_in", [128, ES], bf16, kind="ExternalInput")
    oc1_in = nc.dram_tensor("oc1_in", [72, ES], bf16, kind="ExternalInput")
    oc2_in = nc.dram_tensor("oc2_in", [128, ES], bf16, kind="ExternalInput")
    oc3_in = nc.dram_tensor("oc3_in", [72, ES], bf16, kind="ExternalInput")
    bnp_in = nc.dram_tensor("bnp_in", [ED, 8], f32, kind="ExternalInput")  # g11,b11,g12,b12,g21,b21,g22,b22
    out_t = nc.dram_tensor("out", [B, ES], bf16, kind="ExternalOutput")

    FS = [(0, 128), (128, 72)]  # feat M-tiles (offset, len)

    with tile.TileContext(nc) as tc:
        from contextlib import ExitStack
        with ExitStack() as ctx:
            singles = ctx.enter_context(tc.tile_pool(name="singles", bufs=1))
            xpool = ctx.enter_context(tc.tile_pool(name="xpool", bufs=1))
            small = ctx.enter_context(tc.tile_pool(name="small", bufs=4))
            btmp = ctx.enter_context(tc.tile_pool(name="btmp", bufs=2))
            perst = ctx.enter_context(tc.tile_pool(name="perst", bufs=1))
            opool = ctx.enter_context(tc.tile_pool(name="ocat", bufs=1))
            dram = ctx.enter_context(tc.tile_pool(name="dram", bufs=1, space="DRAM"))

            # BN params in both partition alignments
            bnp100 = singles.tile([100, 2, 8], f32)
            nc.sync.dma_start(bnp100[:], bnp_in.rearrange("(h p) c -> p h c", p=100))
            bnpA = singles.tile([128, 8], f32)
            nc.sync.dma_start(bnpA[:], bnp_in[0:128, :])
            bnpB = singles.tile([72, 8], f32)
            nc.sync.dma_start(bnpB[:], bnp_in[128:200, :])
            eps100 = singles.tile([100, 1], f32)
            nc.vector.memset(eps100, BN_EPS)
            eps128 = singles.tile([128, 1], f32)
            nc.vector.memset(eps128, BN_EPS)

            def bn_normalize(src_ap, dst_tile, gcol, bcol, par_ap, eps_tile,
                             postmul=None, premul=None):
                """dst = BN(src [* premul]) * g + b [* postmul] — batch stats along free dim.
                Apply runs on the scalar engine; stats on vector."""
                P = dst_tile.shape[0]
                if premul is not None:
                    pre = btmp.tile([128, B], f32, tag="bn_pre", bufs=1)
                    nc.vector.tensor_tensor(pre[:P], src_ap, premul, mybir.AluOpType.mult)
                    src_ap = pre[:P]
                stats = small.tile([128, 4, 6], f32, tag="bn_stats")
                for i in range(4):
                    nc.vector.bn_stats(stats[:P, i, :], src_ap[:, i * 512:(i + 1) * 512])
                mv = small.tile([128, 2], f32, tag="bn_mv")
                nc.vector.bn_aggr(mv[:P], stats[:P])
                rstd = small.tile([128, 1], f32, tag="bn_rstd")
                nc.scalar.activation(rstd[:P], mv[:P, 1:2], AF.Sqrt,
                                     bias=eps_tile[:P], scale=1.0)
                nc.vector.reciprocal(rstd[:P], rstd[:P])
                A = small.tile([128, 1], f32, tag="bn_A")
                nc.vector.tensor_mul(A[:P], rstd[:P], gcol)
                Bt = small.tile([128, 1], f32, tag="bn_B")
                nc.vector.tensor_mul(Bt[:P], mv[:P, 0:1], A[:P])
                nc.vector.tensor_tensor(Bt[:P], bcol, Bt[:P], mybir.AluOpType.subtract)
                nc.scalar.activation(dst_tile[:], src_ap, AF.Identity,
                                     bias=Bt[:P], scale=A[:P])
                if postmul is not None:
                    nc.vector.tensor_tensor(dst_tile[:], dst_tile[:], postmul,
                                            mybir.AluOpType.mult)

            def load_chunked(dst, src, nchunk=4):
                step = B // nchunk
                for i in range(nchunk):
                    nc.sync.dma_start(dst[:, i * step:(i + 1) * step],
                                      src[:, i * step:(i + 1) * step])

            # ---------- input BN: x1 first (branch 1 starts immediately);
            # x2/ts BN is issued mid-branch-1 (see below) ----------
            x1t = []
            for h in range(2):
                raw1 = btmp.tile([100, B], bf16, tag="raw_in")
                load_chunked(raw1[:], x1_in[100 * h:100 * (h + 1), :])
                d1 = xpool.tile([100, B], bf16, name=f"x1t_{h}")
                bn_normalize(raw1[:], d1, bnp100[:, h, 0:1], bnp100[:, h, 1:2], bnp100, eps100)
                x1t.append(d1)

            m1t = []

            def build_m1t():
                for h in range(2):
                    raw2 = btmp.tile([100, B], bf16, tag="raw_in")
                    load_chunked(raw2[:], x2_in[100 * h:100 * (h + 1), :])
                    tsh = btmp.tile([100, B], bf16, tag="ts_in")
                    load_chunked(tsh[:], ts_in[100 * h:100 * (h + 1), :])
                    d2 = xpool.tile([100, B], bf16, name=f"m1t_{h}")
                    bn_normalize(raw2[:], d2, bnp100[:, h, 4:5], bnp100[:, h, 5:6],
                                 bnp100, eps100, postmul=tsh[:])
                    m1t.append(d2)

            # ---------- Tucker branches ----------
            # per (branch, half) AR buffers, all bf16. br=1: branch1, br=0: branch2
            pe_dram = {}
            pe_out_dram = {}
            for br in range(2):
                for hf in range(2):
                    pe_dram[(br, hf)] = dram.tile([ED, B], bf16, name=f"pe_{br}_{hf}")
                    pe_out_dram[(br, hf)] = dram.tile([ED, B], bf16, addr_space="Shared",
                                                      name=f"peo_{br}_{hf}")

            # pb broadcast chunking: branch1 r0-2 via gpsimd (fast start), the rest
            # via batched broadcast DMAs (4 r's per chunk), prefetched a phase ahead
            QR = 4
            GPS_R = 3          # branch1 r's served by gpsimd broadcast

            with tc.tile_pool(name="tucker", bufs=3) as tpool, \
                 tc.tile_pool(name="gw", bufs=4) as gwpool, \
                 tc.tile_pool(name="pbrow", bufs=3) as prow, \
                 tc.tile_pool(name="pbbc", bufs=3) as pbc, \
                 tc.tile_pool(name="pbq", bufs=3) as pbq, \
                 tc.tile_pool(name="psum_tk", bufs=1, space="PSUM") as pst:

                g1_sb = singles.tile([100, RS, 2, ED], bf16)
                for r in range(RS):
                    nc.sync.dma_start(g1_sb[:, r:r + 1], g1_in[:, r:r + 1])

                pb_src = {}       # (br, r) -> AP [100, B]

                def issue_pb_chunk(br, c0, c1):
                    pbin = pb2_in if br == 0 else pb1_in
                    t = pbq.tile([100, QR * B], bf16, tag="pbq")
                    n = c1 - c0
                    src = pbin[c0:c1, :].rearrange("r b -> (r b)")[None, :]
                    for q in range(4):   # 4-way partition split -> parallel queues
                        nc.sync.dma_start(
                            t[25 * q:25 * (q + 1), 0:n * B],
                            src.partition_broadcast(25).squeeze(1))
                    for r in range(c0, c1):
                        pb_src[(br, r)] = t[:, (r - c0) * B:(r - c0 + 1) * B]

                def issue_pb_gps(br, r):
                    pbin = pb2_in if br == 0 else pb1_in
                    row = prow.tile([1, B], bf16, tag="pbrow")
                    nc.sync.dma_start(row[:], pbin[r:r + 1, :])
                    pb = pbc.tile([100, B], bf16, tag="pbbc")
                    nc.gpsimd.partition_broadcast(pb[:], row[0:1, :], channels=100)
                    pb_src[(br, r)] = pb[:]

                # branch1 start: 3 gpsimd broadcasts + first DMA chunks
                for r in range(GPS_R):
                    issue_pb_gps(1, r)
                issue_pb_chunk(1, 3, 7)
                issue_pb_chunk(1, 7, 11)

                def tucker_half(br, r0, r1, sched=None):
                    """br=1: branch1 (resident G1); br=0: branch2 (streamed G2 pieces)."""
                    ps_a = pst.tile([128, B], f32, tag="ps_m0", name=f"ps_{br}_{r0}_a")
                    ps_b = pst.tile([72, B], f32, tag="ps_m1", name=f"ps_{br}_{r0}_b")
                    ps = [ps_a, ps_b]
                    xt = m1t if br == 0 else x1t
                    if br == 0:
                        # piece-wise column coverage: zero + accumulate-only
                        nc.scalar.memzero(ps_a[:])
                        nc.scalar.memzero(ps_b[:])
                    for r in range(r0, r1):
                        if sched and r in sched:
                            issue_pb_chunk(*sched[r])
                        pb = pb_src[(br, r)]
                        for h in range(2):
                            first = (r == r0 and h == 0)
                            last = (r == r1 - 1 and h == 1)
                            z = tpool.tile([100, B], bf16, tag="z")
                            nc.vector.tensor_tensor(z[:], xt[h][:], pb,
                                                    mybir.AluOpType.mult)
                            if br == 0:
                                g2c = gwpool.tile([100, C, ED], bf16, tag="g2w")
                                for q in range(4):
                                    nc.sync.dma_start(
                                        g2c[:, 3 * q:3 * (q + 1)],
                                        g2_in[r, h, :, 3 * q:3 * (q + 1)])
                                for mi, (mo, ml) in enumerate(FS):
                                    for (cid, off, ln) in pieces:
                                        nc.tensor.matmul(
                                            ps[mi][:, off:off + ln],
                                            lhsT=g2c[:, cid, mo:mo + ml],
                                            rhs=z[:, off:off + ln],
                                            start=False, stop=last,
                                            skip_group_check=True)
                            else:
                                for mi, (mo, ml) in enumerate(FS):
                                    for bc in range(4):
                                        nc.tensor.matmul(
                                            ps[mi][:, bc * 512:(bc + 1) * 512],
                                            lhsT=g1_sb[:, r, h, mo:mo + ml],
                                            rhs=z[:, bc * 512:(bc + 1) * 512],
                                            start=first, stop=last)
                    # evict on scalar engine + AllReduce this half
                    hf = 0 if r0 == 0 else 1
                    for mi, (mo, ml) in enumerate(FS):
                        ev = btmp.tile([128, B], bf16, tag="pe_evict")
                        nc.scalar.copy(ev[:ml], ps[mi][:])
                        nc.sync.dma_start(pe_dram[(br, hf)][mo:mo + ml, 0:B // 2],
                                          ev[:ml, 0:B // 2])
                        nc.sync.dma_start(pe_dram[(br, hf)][mo:mo + ml, B // 2:B],
                                          ev[:ml, B // 2:B])
                    nc.gpsimd.collective_compute(
                        "AllReduce", mybir.AluOpType.add,
                        replica_groups=[list(range(NCORES))],
                        ins=[pe_dram[(br, hf)].opt()],
                        outs=[pe_out_dram[(br, hf)].opt()])

                # branch1 halves with staggered pb-chunk prefetch (<=3 chunk tiles
                # outstanding at any time = pbq bufs); x2-BN lands mid-branch-1
                tucker_half(1, 0, 13, sched={5: (1, 11, 15), 10: (1, 15, 19)})
                build_m1t()
                tucker_half(1, 13, 25, sched={14: (1, 19, 23), 17: (1, 23, 25),
                                              19: (0, 0, 4), 22: (0, 4, 8)})
                tucker_half(0, 0, 13, sched={2: (0, 8, 12), 6: (0, 12, 16),
                                             10: (0, 16, 20)})
                tucker_half(0, 13, 25, sched={14: (0, 20, 24), 17: (0, 24, 25)})

            # O tiles load after tucker SBUF frees (overlaps AR/post chain)
            oc_sb = []
            for i, (oin, P) in enumerate([(oc0_in, 128), (oc1_in, 72),
                                          (oc2_in, 128), (oc3_in, 72)]):
                t = opool.tile([P, ES], bf16, name=f"oc_{i}")
                nc.sync.dma_start(t[:, 0:ES // 2], oin[:, 0:ES // 2])
                nc.sync.dma_start(t[:, ES // 2:ES], oin[:, ES // 2:ES])
                oc_sb.append(t)

            # ---------- post: readback (DMA-accumulated) + BN ----------
            pe_bn_map = {}
            with tc.tile_pool(name="postp", bufs=2) as postp:
                for br in (1, 0):   # branch1's ARs finish first
                    for mi, (mo, ml) in enumerate(FS):
                        rA = postp.tile([128, B], bf16, tag="pe_rA")
                        nc.sync.dma_start(rA[:ml], pe_out_dram[(br, 0)][mo:mo + ml, :])
                        rB = postp.tile([128, B], bf16, tag="pe_rB")
                        nc.sync.dma_start(rB[:ml], pe_out_dram[(br, 1)][mo:mo + ml, :])
                        sm = postp.tile([128, B], bf16, tag="pe_sum")
                        nc.gpsimd.tensor_tensor(sm[:ml], rA[:ml], rB[:ml],
                                                mybir.AluOpType.add)
                        extra = None
                        if br == 0:
                            toh = postp.tile([128, B], bf16, tag="to_in")
                            nc.sync.dma_start(toh[:ml], to_in[mo:mo + ml, :])
                            extra = toh[:ml]
                        par = bnpA if mi == 0 else bnpB
                        # branch2 uses g22/b22 (cols 6,7); branch1 uses g12/b12 (cols 2,3)
                        cbase = 6 if br == 0 else 2
                        dst = perst.tile([128, B], bf16, name=f"pebn_{br}_{mi}")
                        bn_normalize(sm[:ml], dst[:ml], par[:, cbase:cbase + 1],
                                     par[:, cbase + 1:cbase + 2], par, eps128, premul=extra)
                        pe_bn_map[(br, mi)] = (dst, ml)

            # K-tile order for logits: br1_m0, br1_m1, br2_m0, br2_m1 matching oc order
            pe_bn = [pe_bn_map[(1, 0)], pe_bn_map[(1, 1)],
                     pe_bn_map[(0, 0)], pe_bn_map[(0, 1)]]

            # ---------- logits matmul + sigmoid + store ----------
            with tc.tile_pool(name="logits", bufs=4) as lpool, \
                 tc.tile_pool(name="psum_l", bufs=8, space="PSUM") as psl:
                for mb in range(B // 128):
                    orow = lpool.tile([128, ES], bf16, tag="orow", bufs=4)
                    for vc in range(ES // 500):
                        psu = psl.tile([128, 512], f32, tag="ps_l")
                        for kc in range(4):
                            peb, kl = pe_bn[kc]
                            nc.tensor.matmul(
                                psu[:, 0:500],
                                lhsT=peb[:kl, mb * 128:(mb + 1) * 128],
                                rhs=oc_sb[kc][:, vc * 500:(vc + 1) * 500],
                                start=(kc == 0), stop=(kc == 3))
                        nc.scalar.activation(orow[:, vc * 500:(vc + 1) * 500], psu[:, 0:500],
                                             AF.Sigmoid)
                    for q in range(4):   # chunked store -> parallel queues
                        qs = ES // 4
                        nc.sync.dma_start(
                            out_t[mb * 128:(mb + 1) * 128, q * qs:(q + 1) * qs],
                            orow[:, q * qs:(q + 1) * qs])

    nc.compile()
    return nc


def kernel(s, p, o, times, fine2coarse, S1, O1, S2, O2, P1, P2, G1, G2, T_S, T_O,
           g11, b11, g12, b12, g21, b21, g22, b22):
    from concourse.bass_utils import run_bass_kernel_spmd

    s = np.asarray(s); p = np.asarray(p); times = np.asarray(times)
    fine2coarse = np.asarray(fine2coarse)

    # ----- host-side routing (index logistics only) -----
    c = fine2coarse[times]                       # [B] coarse id per sample
    perm = np.argsort(c, kind="stable")
    c_sorted = c[perm]
    counts = np.bincount(c_sorted, minlength=C)
    offs = np.concatenate([[0], np.cumsum(counts)])
    pieces = []
    for cid in range(C):
        pos, en = int(offs[cid]), int(offs[cid + 1])
        while pos < en:
            nxt = min(en, pos + 512)     # moving-operand cap; PSUM straddle is fine
            pieces.append((cid, pos, nxt - pos))
            pos = nxt
    pieces = tuple(pieces)

    key = pieces
    if key not in _cache:
        _cache[key] = _build(pieces)
    nc = _cache[key]

    s_p, p_p, t_p = s[perm], p[perm], times[perm]

    def bt(x):
        return np.ascontiguousarray(x, dtype=BF16)

    x1_in = bt(np.asarray(S1)[s_p].T)
    x2_in = bt(np.asarray(S2)[s_p].T)
    ts_in = bt(np.asarray(T_S)[t_p].T)
    to_in = bt(np.asarray(T_O)[t_p].T)
    pp1 = np.asarray(P1)[p_p]                       # [B, RD]
    pp2 = np.asarray(P2)[p_p]
    G1 = np.asarray(G1); G2 = np.asarray(G2)
    O1 = np.asarray(O1); O2 = np.asarray(O2)
    bnp = np.stack([g11, b11, g12, b12, g21, b21, g22, b22], axis=1).astype(np.float32)
    bnp = np.ascontiguousarray(bnp)

    in_maps = []
    for k in range(NCORES):
        rs = slice(RS * k, RS * (k + 1))
        vs = slice(ES * k, ES * (k + 1))
        g1k = bt(G1[rs].reshape(RS, 2, 100, ED).transpose(2, 0, 1, 3))
        g2k = bt(G2[:, rs].reshape(C, RS, 2, 100, ED).transpose(1, 2, 3, 0, 4))
        pb1 = bt(pp1[:, rs].T)
        pb2 = bt(pp2[:, rs].T)
        o1t = bt(O1[vs].T)   # [200, ES]
        o2t = bt(O2[vs].T)
        in_maps.append({
            "x1_in": x1_in, "x2_in": x2_in, "ts_in": ts_in, "to_in": to_in,
            "g1_in": g1k, "g2_in": g2k, "pb1_in": pb1, "pb2_in": pb2,
            "oc0_in": np.ascontiguousarray(o1t[0:128]),
            "oc1_in": np.ascontiguousarray(o1t[128:200]),
            "oc2_in": np.ascontiguousarray(o2t[0:128]),
            "oc3_in": np.ascontiguousarray(o2t[128:200]),
            "bnp_in": bnp,
        })

    res = run_bass_kernel_spmd(nc, in_maps, core_ids=list(range(NCORES)))

    out_sorted = np.concatenate(
        [res.results[k]["out"].astype(np.float32) for k in range(NCORES)], axis=1)
    out = np.empty_like(out_sorted)
    out[perm] = out_sorted
    return out


# revision 19
# speedup vs baseline: 1.1464x; 1.1464x over previous
# kernel.py — self-contained Trainium2 Bass kernel for nn_BTDG_31774168055963 (moe_routing)
#
# Reference computation:
#   branch1: x1 = BN(S1[s]); pe1 = einsum('be,bef->bf', x1, (P1[p] @ G1.reshape(rd,ed*ed)).reshape(-1,ed,ed))
#            pe1 = BN(pe1); pred1 = pe1 @ O1.T
#   branch2: x2 = BN(S2[s]); m1 = x2*T_S[times]; per-coarse-time Tucker core G2[c]
#            pe2 = sum_c [c==fine2coarse[times]] einsum(m1, (P2[p] @ G2[c].reshape(rd,ed*ed)).reshape(-1,ed,ed))
#            pe2 = BN(pe2 * T_O[times]); pred2 = pe2 @ O2.T
#   out = sigmoid(pred1 + pred2)
#
# Strategy (8 NeuronCores):
#   - shard Tucker rank rd=200 -> 25 per core; host sorts samples by coarse bucket
#   - Tucker via outer-product moving operand Z[(e),b] = pp[r,b]*x[e,b]
#   - pp replication: branch1 via per-r DMA broadcast (DMA is otherwise idle there),
#     branch2 via gpsimd.partition_broadcast (DMA is saturated by the G2 stream)
#   - branch1 (resident G1) first; G2 streams during/into branch2; each branch split
#     into two r-halves whose partial pe is AllReduced (bf16) behind the next slab
#   - PSUM evictions, zeros and BN applies on the scalar engine; half-sums on gpsimd;
#     vector keeps z-multiplies and BN statistics
#   - logits matmul sharded column-wise over E=20000 -> 2500/core; sigmoid bf16 out

import numpy as np
import ml_dtypes

BF16 = ml_dtypes.bfloat16

B, E, R2, T, C, ED, RD = 2048, 20000, 500, 365, 12, 200, 200
NCORES = 8
RS = RD // NCORES       # 25 r's per core
ES = E // NCORES        # 2500 vocab per core
BN_EPS = 1e-5
RHALVES = [(0, 13), (13, 25)]

_cache = {}


def _build(pieces):
    """Build + compile the per-core bass kernel. `pieces` is a tuple of
    (coarse_id, col_off, col_len) for branch-2 bucket matmuls."""
    import concourse.bass as bass
    import concourse.mybir as mybir
    import concourse.tile as tile
    from concourse import bacc

    f32 = mybir.dt.float32
    bf16 = mybir.dt.bfloat16
    AF = mybir.ActivationFunctionType

    nc = bacc.Bacc("TRN2", target_bir_lowering=False, debug=False, num_devices=NCORES)

    # ---------------- I/O ----------------
    x1_in = nc.dram_tensor("x1_in", [ED, B], bf16, kind="ExternalInput")   # S1[s_p].T
    x2_in = nc.dram_tensor("x2_in", [ED, B], bf16, kind="ExternalInput")   # S2[s_p].T
    ts_in = nc.dram_tensor("ts_in", [ED, B], bf16, kind="ExternalInput")   # T_S[times_p].T
    to_in = nc.dram_tensor("to_in", [ED, B], bf16, kind="ExternalInput")   # T_O[times_p].T
    g1_in = nc.dram_tensor("g1_in", [100, RS, 2, ED], bf16, kind="ExternalInput")
    g2_in = nc.dram_tensor("g2_in", [RS, 2, 100, C, ED], bf16, kind="ExternalInput")
    pb1_in = nc.dram_tensor("pb1_in", [RS, B], bf16, kind="ExternalInput")  # P1[p_p].T r-slice
    pb2_in = nc.dram_tensor("pb2_in", [RS, B], bf16, kind="ExternalInput")
    # O chunks: feat layout {0:128, 128:200} x {O1, O2}
    oc0_in = nc.dram_tensor("oc0_in", [128, ES], bf16, kind="ExternalInput")
    oc1_in = nc.dram_tensor("oc1_in", [72, ES], bf16, kind="ExternalInput")
    oc2_in = nc.dram_tensor("oc2_in", [128, ES], bf16, kind="ExternalInput")
    oc3_in = nc.dram_tensor("oc3_in", [72, ES], bf16, kind="ExternalInput")
    bnp_in = nc.dram_tensor("bnp_in", [ED, 8], f32, kind="ExternalInput")  # g11,b11,g12,b12,g21,b21,g22,b22
    out_t = nc.dram_tensor("out", [B, ES], bf16, kind="ExternalOutput")

    FS = [(0, 128), (128, 72)]  # feat M-tiles (offset, len)

    with tile.TileContext(nc) as tc:
        from contextlib import ExitStack
        with ExitStack() as ctx:
            singles = ctx.enter_context(tc.tile_pool(name="singles", bufs=1))
            xpool = ctx.enter_context(tc.tile_pool(name="xpool", bufs=1))
            small = ctx.enter_context(tc.tile_pool(name="small", bufs=4))
            btmp = ctx.enter_context(tc.tile_pool(name="btmp", bufs=2))
            perst = ctx.enter_context(tc.tile_pool(name="perst", bufs=1))
            opool = ctx.enter_context(tc.tile_pool(name="ocat", bufs=1))
            dram = ctx.enter_context(tc.tile_pool(name="dram", bufs=1, space="DRAM"))

            # BN params in both partition alignments
            bnp100 = singles.tile([100, 2, 8], f32)
            nc.sync.dma_start(bnp100[:], bnp_in.rearrange("(h p) c -> p h c", p=100))
            bnpA = singles.tile([128, 8], f32)
            nc.sync.dma_start(bnpA[:], bnp_in[0:128, :])
            bnpB = singles.tile([72, 8], f32)
            nc.sync.dma_start(bnpB[:], bnp_in[128:200, :])
            eps100 = singles.tile([100, 1], f32)
            nc.vector.memset(eps100, BN_EPS)
            eps128 = singles.tile([128, 1], f32)
            nc.vector.memset(eps128, BN_EPS)

            def bn_normalize(src_ap, dst_tile, gcol, bcol, par_ap, eps_tile,
                             postmul=None, premul=None):
                """dst = BN(src [* premul]) * g + b [* postmul] — batch stats along free dim.
                Apply runs on the scalar engine; stats on vector."""
                P = dst_tile.shape[0]
                if premul is not None:
                    pre = btmp.tile([128, B], f32, tag="bn_pre", bufs=1)
                    nc.vector.tensor_tensor(pre[:P], src_ap, premul, mybir.AluOpType.mult)
                    src_ap = pre[:P]
                stats = small.tile([128, 4, 6], f32, tag="bn_stats")
                for i in range(4):
                    nc.vector.bn_stats(stats[:P, i, :], src_ap[:, i * 512:(i + 1) * 512])
                mv = small.tile([128, 2], f32, tag="bn_mv")
                nc.vector.bn_aggr(mv[:P], stats[:P])
                rstd = small.tile([128, 1], f32, tag="bn_rstd")
                nc.scalar.activation(rstd[:P], mv[:P, 1:2], AF.Sqrt,
                                     bias=eps_tile[:P], scale=1.0)
                nc.vector.reciprocal(rstd[:P], rstd[:P])
                A = small.tile([128, 1], f32, tag="bn_A")
                nc.vector.tensor_mul(A[:P], rstd[:P], gcol)
                Bt = small.tile([128, 1], f32, tag="bn_B")
                nc.vector.tensor_mul(Bt[:P], mv[:P, 0:1], A[:P])
                nc.vector.tensor_tensor(Bt[:P], bcol, Bt[:P], mybir.AluOpType.subtract)
                nc.scalar.activation(dst_tile[:], src_ap, AF.Identity,
                                     bias=Bt[:P], scale=A[:P])
                if postmul is not None:
                    nc.vector.tensor_tensor(dst_tile[:], dst_tile[:], postmul,
                                            mybir.AluOpType.mult)

            def load_chunked(dst, src, nchunk=4):
                step = B // nchunk
                for i in range(nchunk):
                    nc.sync.dma_start(dst[:, i * step:(i + 1) * step],
                                      src[:, i * step:(i + 1) * step])

            # ---------- input BN: x1 first (branch 1 starts immediately) ----------
            x1t = []
            for h in range(2):
                raw1 = btmp.tile([100, B], bf16, tag="raw_in")
                load_chunked(raw1[:], x1_in[100 * h:100 * (h + 1), :])
                d1 = xpool.tile([100, B], bf16, name=f"x1t_{h}")
                bn_normalize(raw1[:], d1, bnp100[:, h, 0:1], bnp100[:, h, 1:2], bnp100, eps100)
                x1t.append(d1)

            m1t = []

            def build_m1t():
                for h in range(2):
                    raw2 = btmp.tile([100, B], bf16, tag="raw_in")
                    load_chunked(raw2[:], x2_in[100 * h:100 * (h + 1), :])
                    tsh = btmp.tile([100, B], bf16, tag="ts_in")
                    load_chunked(tsh[:], ts_in[100 * h:100 * (h + 1), :])
                    d2 = xpool.tile([100, B], bf16, name=f"m1t_{h}")
                    bn_normalize(raw2[:], d2, bnp100[:, h, 4:5], bnp100[:, h, 5:6],
                                 bnp100, eps100, postmul=tsh[:])
                    m1t.append(d2)

            # ---------- Tucker branches ----------
            pe_dram = {}
            pe_out_dram = {}
            for br in range(2):
                for hf in range(2):
                    pe_dram[(br, hf)] = dram.tile([ED, B], bf16, name=f"pe_{br}_{hf}")
                    pe_out_dram[(br, hf)] = dram.tile([ED, B], bf16, addr_space="Shared",
                                                      name=f"peo_{br}_{hf}")

            with tc.tile_pool(name="tucker", bufs=4) as tpool, \
                 tc.tile_pool(name="pbd", bufs=5) as pbd, \
                 tc.tile_pool(name="gw", bufs=7) as gwpool, \
                 tc.tile_pool(name="pbrow", bufs=3) as prow, \
                 tc.tile_pool(name="pbbc", bufs=4) as pbc, \
                 tc.tile_pool(name="psum_tk", bufs=1, space="PSUM") as pst:

                g1_sb = singles.tile([100, RS, 2, ED], bf16)
                for r in range(RS):
                    nc.sync.dma_start(g1_sb[:, r:r + 1], g1_in[:, r:r + 1])

                def get_pb(br, r):
                    if br == 1:
                        # branch1: per-r DMA broadcast, 2-way split for latency
                        pb = pbd.tile([100, B], bf16, tag="pbdma")
                        src = pb1_in[r:r + 1, :]
                        nc.sync.dma_start(
                            pb[0:50], src.partition_broadcast(50).squeeze(1))
                        nc.sync.dma_start(
                            pb[50:100], src.partition_broadcast(50).squeeze(1))
                        return pb[:]
                    # branch2: gpsimd broadcast (DMA busy with G2)
                    row = prow.tile([1, B], bf16, tag="pbrow")
                    nc.sync.dma_start(row[:], pb2_in[r:r + 1, :])
                    pb = pbc.tile([100, B], bf16, tag="pbbc")
                    nc.gpsimd.partition_broadcast(pb[:], row[0:1, :], channels=100)
                    return pb[:]

                def tucker_half(br, r0, r1):
                    """br=1: branch1 (resident G1); br=0: branch2 (streamed G2 pieces)."""
                    ps_a = pst.tile([128, B], f32, tag="ps_m0", name=f"ps_{br}_{r0}_a")
                    ps_b = pst.tile([72, B], f32, tag="ps_m1", name=f"ps_{br}_{r0}_b")
                    ps = [ps_a, ps_b]
                    xt = m1t if br == 0 else x1t
                    if br == 0:
                        # piece-wise column coverage: zero + accumulate-only
                        nc.scalar.memzero(ps_a[:])
                        nc.scalar.memzero(ps_b[:])
                    for r in range(r0, r1):
                        pb = get_pb(br, r)
                        for h in range(2):
                            first = (r == r0 and h == 0)
                            last = (r == r1 - 1 and h == 1)
                            z = tpool.tile([100, B], bf16, tag="z")
                            nc.vector.tensor_tensor(z[:], xt[h][:], pb,
                                                    mybir.AluOpType.mult)
                            if br == 0:
                                g2c = gwpool.tile([100, C, ED], bf16, tag="g2w")
                                for q in range(4):
                                    nc.sync.dma_start(
                                        g2c[:, 3 * q:3 * (q + 1)],
                                        g2_in[r, h, :, 3 * q:3 * (q + 1)])
                                for mi, (mo, ml) in enumerate(FS):
                                    for (cid, off, ln) in pieces:
                                        nc.tensor.matmul(
                                            ps[mi][:, off:off + ln],
                                            lhsT=g2c[:, cid, mo:mo + ml],
                                            rhs=z[:, off:off + ln],
                                            start=False, stop=last,
                                            skip_group_check=True)
                            else:
                                for mi, (mo, ml) in enumerate(FS):
                                    for bc in range(4):
                                        nc.tensor.matmul(
                                            ps[mi][:, bc * 512:(bc + 1) * 512],
                                            lhsT=g1_sb[:, r, h, mo:mo + ml],
                                            rhs=z[:, bc * 512:(bc + 1) * 512],
                                            start=first, stop=last)
                    # evict on scalar engine + AllReduce this half
                    hf = 0 if r0 == 0 else 1
                    for mi, (mo, ml) in enumerate(FS):
                        ev = btmp.tile([128, B], bf16, tag="pe_evict")
                        nc.scalar.copy(ev[:ml], ps[mi][:])
                        nc.sync.dma_start(pe_dram[(br, hf)][mo:mo + ml, 0:B // 2],
                                          ev[:ml, 0:B // 2])
                        nc.sync.dma_start(pe_dram[(br, hf)][mo:mo + ml, B // 2:B],
                                          ev[:ml, B // 2:B])
                    nc.gpsimd.collective_compute(
                        "AllReduce", mybir.AluOpType.add,
                        replica_groups=[list(range(NCORES))],
                        ins=[pe_dram[(br, hf)].opt()],
                        outs=[pe_out_dram[(br, hf)].opt()])

                tucker_half(1, 0, 13)
                build_m1t()          # x2/ts load + BN lands mid-branch-1 on vector
                tucker_half(1, 13, 25)
                tucker_half(0, 0, 13)
                tucker_half(0, 13, 25)

            # O tiles load after tucker SBUF frees (overlaps AR/post chain)
            oc_sb = []
            for i, (oin, P) in enumerate([(oc0_in, 128), (oc1_in, 72),
                                          (oc2_in, 128), (oc3_in, 72)]):
                t = opool.tile([P, ES], bf16, name=f"oc_{i}")
                nc.sync.dma_start(t[:, 0:ES // 2], oin[:, 0:ES // 2])
                nc.sync.dma_start(t[:, ES // 2:ES], oin[:, ES // 2:ES])
                oc_sb.append(t)

            # ---------- post: readback + add halves (gpsimd) + BN ----------
            pe_bn_map = {}
            with tc.tile_pool(name="postp", bufs=2) as postp:
                for br in (1, 0):   # branch1's ARs finish first
                    for mi, (mo, ml) in enumerate(FS):
                        rA = postp.tile([128, B], bf16, tag="pe_rA")
                        nc.sync.dma_start(rA[:ml], pe_out_dram[(br, 0)][mo:mo + ml, :])
                        rB = postp.tile([128, B], bf16, tag="pe_rB")
                        nc.sync.dma_start(rB[:ml], pe_out_dram[(br, 1)][mo:mo + ml, :])
                        sm = postp.tile([128, B], bf16, tag="pe_sum")
                        nc.gpsimd.tensor_tensor(sm[:ml], rA[:ml], rB[:ml],
                                                mybir.AluOpType.add)
                        extra = None
                        if br == 0:
                            toh = postp.tile([128, B], bf16, tag="to_in")
                            nc.sync.dma_start(toh[:ml], to_in[mo:mo + ml, :])
                            extra = toh[:ml]
                        par = bnpA if mi == 0 else bnpB
                        # branch2 uses g22/b22 (cols 6,7); branch1 uses g12/b12 (cols 2,3)
                        cbase = 6 if br == 0 else 2
                        dst = perst.tile([128, B], bf16, name=f"pebn_{br}_{mi}")
                        bn_normalize(sm[:ml], dst[:ml], par[:, cbase:cbase + 1],
                                     par[:, cbase + 1:cbase + 2], par, eps128, premul=extra)
                        pe_bn_map[(br, mi)] = (dst, ml)

            # K-tile order for logits: br1_m0, br1_m1, br2_m0, br2_m1 matching oc order
            pe_bn = [pe_bn_map[(1, 0)], pe_bn_map[(1, 1)],
                     pe_bn_map[(0, 0)], pe_bn_map[(0, 1)]]

            # ---------- logits matmul + sigmoid + store ----------
            with tc.tile_pool(name="logits", bufs=4) as lpool, \
                 tc.tile_pool(name="psum_l", bufs=8, space="PSUM") as psl:
                for mb in range(B // 128):
                    orow = lpool.tile([128, ES], bf16, tag="orow", bufs=4)
                    for vc in range(ES // 500):
                        psu = psl.tile([128, 512], f32, tag="ps_l")
                        for kc in range(4):
                            peb, kl = pe_bn[kc]
                            nc.tensor.matmul(
                                psu[:, 0:500],
                                lhsT=peb[:kl, mb * 128:(mb + 1) * 128],
                                rhs=oc_sb[kc][:, vc * 500:(vc + 1) * 500],
                                start=(kc == 0), stop=(kc == 3))
                        nc.scalar.activation(orow[:, vc * 500:(vc + 1) * 500], psu[:, 0:500],
                                             AF.Sigmoid)
                    for q in range(4):   # chunked store -> parallel queues
                        qs = ES // 4
                        nc.sync.dma_start(
                            out_t[mb * 128:(mb + 1) * 128, q * qs:(q + 1) * qs],
                            orow[:, q * qs:(q + 1) * qs])

    nc.compile()
    return nc


def kernel(s, p, o, times, fine2coarse, S1, O1, S2, O2, P1, P2, G1, G2, T_S, T_O,
           g11, b11, g12, b12, g21, b21, g22, b22):
    from concourse.bass_utils import run_bass_kernel_spmd

    s = np.asarray(s); p = np.asarray(p); times = np.asarray(times)
    fine2coarse = np.asarray(fine2coarse)

    # ----- host-side routing (index logistics only) -----
    c = fine2coarse[times]                       # [B] coarse id per sample
    perm = np.argsort(c, kind="stable")
    c_sorted = c[perm]
    counts = np.bincount(c_sorted, minlength=C)
    offs = np.concatenate([[0], np.cumsum(counts)])
    pieces = []
    for cid in range(C):
        pos, en = int(offs[cid]), int(offs[cid + 1])
        while pos < en:
            nxt = min(en, pos + 512)     # moving-operand cap; PSUM straddle is fine
            pieces.append((cid, pos, nxt - pos))
            pos = nxt
    pieces = tuple(pieces)

    key = pieces
    if key not in _cache:
        _cache[key] = _build(pieces)
    nc = _cache[key]

    s_p, p_p, t_p = s[perm], p[perm], times[perm]

    def bt(x):
        return np.ascontiguousarray(x, dtype=BF16)

    x1_in = bt(np.asarray(S1)[s_p].T)
    x2_in = bt(np.asarray(S2)[s_p].T)
    ts_in = bt(np.asarray(T_S)[t_p].T)
    to_in = bt(np.asarray(T_O)[t_p].T)
    pp1 = np.asarray(P1)[p_p]                       # [B, RD]
    pp2 = np.asarray(P2)[p_p]
    G1 = np.asarray(G1); G2 = np.asarray(G2)
    O1 = np.asarray(O1); O2 = np.asarray(O2)
    bnp = np.stack([g11, b11, g12, b12, g21, b21, g22, b22], axis=1).astype(np.float32)
    bnp = np.ascontiguousarray(bnp)

    in_maps = []
    for k in range(NCORES):
        rs = slice(RS * k, RS * (k + 1))
        vs = slice(ES * k, ES * (k + 1))
        g1k = bt(G1[rs].reshape(RS, 2, 100, ED).transpose(2, 0, 1, 3))
        g2k = bt(G2[:, rs].reshape(C, RS, 2, 100, ED).transpose(1, 2, 3, 0, 4))
        pb1 = bt(pp1[:, rs].T)
        pb2 = bt(pp2[:, rs].T)
        o1t = bt(O1[vs].T)   # [200, ES]
        o2t = bt(O2[vs].T)
        in_maps.append({
            "x1_in": x1_in, "x2_in": x2_in, "ts_in": ts_in, "to_in": to_in,
            "g1_in": g1k, "g2_in": g2k, "pb1_in": pb1, "pb2_in": pb2,
            "oc0_in": np.ascontiguousarray(o1t[0:128]),
            "oc1_in": np.ascontiguousarray(o1t[128:200]),
            "oc2_in": np.ascontiguousarray(o2t[0:128]),
            "oc3_in": np.ascontiguousarray(o2t[128:200]),
            "bnp_in": bnp,
        })

    res = run_bass_kernel_spmd(nc, in_maps, core_ids=list(range(NCORES)))

    out_sorted = np.concatenate(
        [res.results[k]["out"].astype(np.float32) for k in range(NCORES)], axis=1)
    out = np.empty_like(out_sorted)
    out[perm] = out_sorted
    return out


# revision 23
# speedup vs baseline: 1.2850x; 1.1210x over previous
# kernel.py — self-contained Trainium2 Bass kernel for nn_BTDG_31774168055963 (moe_routing)
#
# Reference computation:
#   branch1: x1 = BN(S1[s]); pe1 = einsum('be,bef->bf', x1, (P1[p] @ G1.reshape(rd,ed*ed)).reshape(-1,ed,ed))
#            pe1 = BN(pe1); pred1 = pe1 @ O1.T
#   branch2: x2 = BN(S2[s]); m1 = x2*T_S[times]; per-coarse-time Tucker core G2[c]
#            pe2 = sum_c [c==fine2coarse[times]] einsum(m1, (P2[p] @ G2[c].reshape(rd,ed*ed)).reshape(-1,ed,ed))
#            pe2 = BN(pe2 * T_O[times]); pred2 = pe2 @ O2.T
#   out = sigmoid(pred1 + pred2)
#
# Strategy (8 NeuronCores):
#   - shard Tucker rank rd=200 -> 25 per core; host sorts samples by coarse bucket
#   - Tucker via outer-product moving operand Z[(e),b] = pp[r,b]*x[e,b]
#   - pp replication: branch1 via per-r DMA broadcast (DMA is otherwise idle there),
#     branch2 via gpsimd.partition_broadcast (DMA is saturated by the G2 stream)
#   - branch1 (resident G1) first; G2 streams during/into branch2; each branch split
#     into two r-halves whose partial pe is AllReduced (bf16) behind the next slab
#   - PSUM evictions, zeros and BN applies on the scalar engine; half-sums on gpsimd;
#     vector keeps z-multiplies and BN statistics
#   - logits matmul sharded column-wise over E=20000 -> 2500/core; sigmoid bf16 out

import numpy as np
import ml_dtypes

BF16 = ml_dtypes.bfloat16

B, E, R2, T, C, ED, RD = 2048, 20000, 500, 365, 12, 200, 200
NCORES = 8
RS = RD // NCORES       # 25 r's per core
ES = E // NCORES        # 2500 vocab per core
BN_EPS = 1e-5
RHALVES = [(0, 13), (13, 25)]

_cache = {}


def _build(pieces):
    """Build + compile the per-core bass kernel. `pieces` is a tuple of
    (coarse_id, col_off, col_len) for branch-2 bucket matmuls."""
    import concourse.bass as bass
    import concourse.mybir as mybir
    import concourse.tile as tile
    from concourse import bacc

    f32 = mybir.dt.float32
    bf16 = mybir.dt.bfloat16
    AF = mybir.ActivationFunctionType

    nc = bacc.Bacc("TRN2", target_bir_lowering=False, debug=False, num_devices=NCORES)

    # ---------------- I/O ----------------
    x1_in = nc.dram_tensor("x1_in", [ED, B], bf16, kind="ExternalInput")   # S1[s_p].T
    x2_in = nc.dram_tensor("x2_in", [ED, B], bf16, kind="ExternalInput")   # S2[s_p].T
    ts_in = nc.dram_tensor("ts_in", [ED, B], bf16, kind="ExternalInput")   # T_S[times_p].T
    to_in = nc.dram_tensor("to_in", [ED, B], bf16, kind="ExternalInput")   # T_O[times_p].T
    g1_in = nc.dram_tensor("g1_in", [100, RS, 2, ED], bf16, kind="ExternalInput")
    g2_in = nc.dram_tensor("g2_in", [RS, 2, 100, C, ED], bf16, kind="ExternalInput")
    pb1_in = nc.dram_tensor("pb1_in", [RS, B], bf16, kind="ExternalInput")  # P1[p_p].T r-slice
    pb2_in = nc.dram_tensor("pb2_in", [RS, B], bf16, kind="ExternalInput")
    # O chunks: feat layout {0:128, 128:200} x {O1, O2}
    oc0_in = nc.dram_tensor("oc0_in", [128, ES], bf16, kind="ExternalInput")
    oc1_in = nc.dram_tensor("oc1_in", [72, ES], bf16, kind="ExternalInput")
    oc2_in = nc.dram_tensor("oc2_in", [128, ES], bf16, kind="ExternalInput")
    oc3_in = nc.dram_tensor("oc3_in", [72, ES], bf16, kind="ExternalInput")
    bnp_in = nc.dram_tensor("bnp_in", [ED, 8], f32, kind="ExternalInput")  # g11,b11,g12,b12,g21,b21,g22,b22
    out_t = nc.dram_tensor("out", [B, ES], bf16, kind="ExternalOutput")

    FS = [(0, 128), (128, 72)]  # feat M-tiles (offset, len)

    with tile.TileContext(nc) as tc:
        from contextlib import ExitStack
        with ExitStack() as ctx:
            singles = ctx.enter_context(tc.tile_pool(name="singles", bufs=1))
            xpool = ctx.enter_context(tc.tile_pool(name="xpool", bufs=1))
            small = ctx.enter_context(tc.tile_pool(name="small", bufs=4))
            btmp = ctx.enter_context(tc.tile_pool(name="btmp", bufs=2))
            perst = ctx.enter_context(tc.tile_pool(name="perst", bufs=1))
            opool = ctx.enter_context(tc.tile_pool(name="ocat", bufs=1))
            dram = ctx.enter_context(tc.tile_pool(name="dram", bufs=1, space="DRAM"))

            # BN params in both partition alignments
            bnp100 = singles.tile([100, 2, 8], f32)
            nc.sync.dma_start(bnp100[:], bnp_in.rearrange("(h p) c -> p h c", p=100))
            bnpA = singles.tile([128, 8], f32)
            nc.sync.dma_start(bnpA[:], bnp_in[0:128, :])
            bnpB = singles.tile([72, 8], f32)
            nc.sync.dma_start(bnpB[:], bnp_in[128:200, :])
            eps100 = singles.tile([100, 1], f32)
            nc.vector.memset(eps100, BN_EPS)
            eps128 = singles.tile([128, 1], f32)
            nc.vector.memset(eps128, BN_EPS)

            def bn_normalize(src_ap, dst_tile, gcol, bcol, par_ap, eps_tile,
                             postmul=None, premul=None):
                """dst = BN(src [* premul]) * g + b [* postmul] — batch stats along free dim.
                Apply runs on the scalar engine; stats on vector."""
                P = dst_tile.shape[0]
                if premul is not None:
                    pre = btmp.tile([128, B], f32, tag="bn_pre", bufs=1)
                    nc.vector.tensor_tensor(pre[:P], src_ap, premul, mybir.AluOpType.mult)
                    src_ap = pre[:P]
                stats = small.tile([128, 4, 6], f32, tag="bn_stats")
                for i in range(4):
                    nc.vector.bn_stats(stats[:P, i, :], src_ap[:, i * 512:(i + 1) * 512])
                mv = small.tile([128, 2], f32, tag="bn_mv")
                nc.vector.bn_aggr(mv[:P], stats[:P])
                rstd = small.tile([128, 1], f32, tag="bn_rstd")
                nc.scalar.activation(rstd[:P], mv[:P, 1:2], AF.Sqrt,
                                     bias=eps_tile[:P], scale=1.0)
                nc.vector.reciprocal(rstd[:P], rstd[:P])
                A = small.tile([128, 1], f32, tag="bn_A")
                nc.vector.tensor_mul(A[:P], rstd[:P], gcol)
                Bt = small.tile([128, 1], f32, tag="bn_B")
                nc.vector.tensor_mul(Bt[:P], mv[:P, 0:1], A[:P])
                nc.vector.tensor_tensor(Bt[:P], bcol, Bt[:P], mybir.AluOpType.subtract)
                nc.scalar.activation(dst_tile[:], src_ap, AF.Identity,
                                     bias=Bt[:P], scale=A[:P])
                if postmul is not None:
                    nc.vector.tensor_tensor(dst_tile[:], dst_tile[:], postmul,
                                            mybir.AluOpType.mult)

            def load_chunked(dst, src, nchunk=4):
                step = B // nchunk
                for i in range(nchunk):
                    nc.sync.dma_start(dst[:, i * step:(i + 1) * step],
                                      src[:, i * step:(i + 1) * step])

            # ---------- input BN: x1 first (branch 1 starts immediately) ----------
            x1t = []
            for h in range(2):
                raw1 = btmp.tile([100, B], bf16, tag="raw_in")
                load_chunked(raw1[:], x1_in[100 * h:100 * (h + 1), :])
                d1 = xpool.tile([100, B], bf16, name=f"x1t_{h}")
                bn_normalize(raw1[:], d1, bnp100[:, h, 0:1], bnp100[:, h, 1:2], bnp100, eps100)
                x1t.append(d1)

            m1t = []

            def build_m1t():
                for h in range(2):
                    raw2 = btmp.tile([100, B], bf16, tag="raw_in")
                    load_chunked(raw2[:], x2_in[100 * h:100 * (h + 1), :])
                    tsh = btmp.tile([100, B], bf16, tag="ts_in")
                    load_chunked(tsh[:], ts_in[100 * h:100 * (h + 1), :])
                    d2 = xpool.tile([100, B], bf16, name=f"m1t_{h}")
                    bn_normalize(raw2[:], d2, bnp100[:, h, 4:5], bnp100[:, h, 5:6],
                                 bnp100, eps100, postmul=tsh[:])
                    m1t.append(d2)

            # ---------- Tucker branches ----------
            pe_dram = {}
            pe_out_dram = {}
            for br in range(2):
                for hf in range(2):
                    pe_dram[(br, hf)] = dram.tile([ED, B], bf16, name=f"pe_{br}_{hf}")
                    pe_out_dram[(br, hf)] = dram.tile([ED, B], bf16, addr_space="Shared",
                                                      name=f"peo_{br}_{hf}")

            oc_sb = []
            with tc.tile_pool(name="tucker", bufs=4) as tpool, \
                 tc.tile_pool(name="pbd", bufs=5) as pbd, \
                 tc.tile_pool(name="gw", bufs=6) as gwpool, \
                 tc.tile_pool(name="pbrow", bufs=3) as prow, \
                 tc.tile_pool(name="pbbc", bufs=4) as pbc, \
                 tc.tile_pool(name="psum_tk", bufs=1, space="PSUM") as pst:

                # G1 + O tiles: issued from the scalar engine's queue (sync is the
                # trigger bottleneck; scalar is idle here)
                g1_sb = singles.tile([100, RS, 2, ED], bf16)
                for r5 in range(5):
                    nc.scalar.dma_start(g1_sb[:, r5 * 5:(r5 + 1) * 5],
                                        g1_in[:, r5 * 5:(r5 + 1) * 5])
                for i, (oin, P) in enumerate([(oc0_in, 128), (oc1_in, 72),
                                              (oc2_in, 128), (oc3_in, 72)]):
                    t = opool.tile([P, ES], bf16, name=f"oc_{i}")
                    nc.scalar.dma_start(t[:], oin[:])
                    oc_sb.append(t)

                def get_pb(br, r):
                    if br == 1:
                        # branch1: per-r DMA broadcast (DMA idle during branch1)
                        pb = pbd.tile([100, B], bf16, tag="pbdma")
                        nc.sync.dma_start(
                            pb[:], pb1_in[r:r + 1, :].partition_broadcast(100).squeeze(1))
                        return pb[:]
                    # branch2: gpsimd broadcast (DMA busy with G2)
                    row = prow.tile([1, B], bf16, tag="pbrow")
                    nc.sync.dma_start(row[:], pb2_in[r:r + 1, :])
                    pb = pbc.tile([100, B], bf16, tag="pbbc")
                    nc.gpsimd.partition_broadcast(pb[:], row[0:1, :], channels=100)
                    return pb[:]

                def tucker_half(br, r0, r1):
                    """br=1: branch1 (resident G1); br=0: branch2 (streamed G2 pieces)."""
                    ps_a = pst.tile([128, B], f32, tag="ps_m0", name=f"ps_{br}_{r0}_a")
                    ps_b = pst.tile([72, B], f32, tag="ps_m1", name=f"ps_{br}_{r0}_b")
                    ps = [ps_a, ps_b]
                    xt = m1t if br == 0 else x1t
                    if br == 0:
                        # piece-wise column coverage: zero + accumulate-only
                        nc.scalar.memzero(ps_a[:])
                        nc.scalar.memzero(ps_b[:])
                    for r in range(r0, r1):
                        pb = get_pb(br, r)
                        for h in range(2):
                            first = (r == r0 and h == 0)
                            last = (r == r1 - 1 and h == 1)
                            z = tpool.tile([100, B], bf16, tag="z")
                            nc.vector.tensor_tensor(z[:], xt[h][:], pb,
                                                    mybir.AluOpType.mult)
                            if br == 0:
                                g2c = gwpool.tile([100, C, ED], bf16, tag="g2w")
                                nc.scalar.dma_start(g2c[:, 0:C // 2],
                                                    g2_in[r, h, :, 0:C // 2])
                                nc.scalar.dma_start(g2c[:, C // 2:C],
                                                    g2_in[r, h, :, C // 2:C])
                                for mi, (mo, ml) in enumerate(FS):
                                    for (cid, off, ln) in pieces:
                                        nc.tensor.matmul(
                                            ps[mi][:, off:off + ln],
                                            lhsT=g2c[:, cid, mo:mo + ml],
                                            rhs=z[:, off:off + ln],
                                            start=False, stop=last,
                                            skip_group_check=True)
                            else:
                                for mi, (mo, ml) in enumerate(FS):
                                    for bc in range(4):
                                        nc.tensor.matmul(
                                            ps[mi][:, bc * 512:(bc + 1) * 512],
                                            lhsT=g1_sb[:, r, h, mo:mo + ml],
                                            rhs=z[:, bc * 512:(bc + 1) * 512],
                                            start=first, stop=last)
                    # evict on scalar engine + AllReduce this half
                    hf = 0 if r0 == 0 else 1
                    for mi, (mo, ml) in enumerate(FS):
                        ev = btmp.tile([128, B], bf16, tag="pe_evict")
                        nc.scalar.copy(ev[:ml], ps[mi][:])
                        nc.sync.dma_start(pe_dram[(br, hf)][mo:mo + ml, 0:B // 2],
                                          ev[:ml, 0:B // 2])
                        nc.sync.dma_start(pe_dram[(br, hf)][mo:mo + ml, B // 2:B],
                                          ev[:ml, B // 2:B])
                    nc.gpsimd.collective_compute(
                        "AllReduce", mybir.AluOpType.add,
                        replica_groups=[list(range(NCORES))],
                        ins=[pe_dram[(br, hf)].opt()],
                        outs=[pe_out_dram[(br, hf)].opt()])

                tucker_half(1, 0, 13)
                build_m1t()          # x2/ts load + BN lands mid-branch-1 on vector
                tucker_half(1, 13, 25)
                tucker_half(0, 0, 13)
                tucker_half(0, 13, 25)

            # ---------- post: readback + add halves (gpsimd) + BN ----------
            pe_bn_map = {}
            with tc.tile_pool(name="postp", bufs=2) as postp:
                for br in (1, 0):   # branch1's ARs finish first
                    for mi, (mo, ml) in enumerate(FS):
                        rA = postp.tile([128, B], bf16, tag="pe_rA")
                        nc.sync.dma_start(rA[:ml], pe_out_dram[(br, 0)][mo:mo + ml, :])
                        rB = postp.tile([128, B], bf16, tag="pe_rB")
                        nc.sync.dma_start(rB[:ml], pe_out_dram[(br, 1)][mo:mo + ml, :])
                        sm = postp.tile([128, B], bf16, tag="pe_sum")
                        nc.gpsimd.tensor_tensor(sm[:ml], rA[:ml], rB[:ml],
                                                mybir.AluOpType.add)
                        extra = None
                        if br == 0:
                            toh = postp.tile([128, B], bf16, tag="to_in")
                            nc.sync.dma_start(toh[:ml], to_in[mo:mo + ml, :])
                            extra = toh[:ml]
                        par = bnpA if mi == 0 else bnpB
                        # branch2 uses g22/b22 (cols 6,7); branch1 uses g12/b12 (cols 2,3)
                        cbase = 6 if br == 0 else 2
                        dst = perst.tile([128, B], bf16, name=f"pebn_{br}_{mi}")
                        bn_normalize(sm[:ml], dst[:ml], par[:, cbase:cbase + 1],
                                     par[:, cbase + 1:cbase + 2], par, eps128, premul=extra)
                        pe_bn_map[(br, mi)] = (dst, ml)

            # K-tile order for logits: br1_m0, br1_m1, br2_m0, br2_m1 matching oc order
            pe_bn = [pe_bn_map[(1, 0)], pe_bn_map[(1, 1)],
                     pe_bn_map[(0, 0)], pe_bn_map[(0, 1)]]

            # ---------- logits matmul + sigmoid + store ----------
            with tc.tile_pool(name="logits", bufs=4) as lpool, \
                 tc.tile_pool(name="psum_l", bufs=8, space="PSUM") as psl:
                for mb in range(B // 128):
                    orow = lpool.tile([128, ES], bf16, tag="orow", bufs=4)
                    for vc in range(ES // 500):
                        psu = psl.tile([128, 512], f32, tag="ps_l")
                        for kc in range(4):
                            peb, kl = pe_bn[kc]
                            nc.tensor.matmul(
                                psu[:, 0:500],
                                lhsT=peb[:kl, mb * 128:(mb + 1) * 128],
                                rhs=oc_sb[kc][:, vc * 500:(vc + 1) * 500],
                                start=(kc == 0), stop=(kc == 3))
                        nc.scalar.activation(orow[:, vc * 500:(vc + 1) * 500], psu[:, 0:500],
                                             AF.Sigmoid)
                    nc.sync.dma_start(out_t[mb * 128:(mb + 1) * 128, :], orow[:])

    nc.compile()
    return nc


def kernel(s, p, o, times, fine2coarse, S1, O1, S2, O2, P1, P2, G1, G2, T_S, T_O,
           g11, b11, g12, b12, g21, b21, g22, b22):
    from concourse.bass_utils import run_bass_kernel_spmd

    s = np.asarray(s); p = np.asarray(p); times = np.asarray(times)
    fine2coarse = np.asarray(fine2coarse)

    # ----- host-side routing (index logistics only) -----
    c = fine2coarse[times]                       # [B] coarse id per sample
    perm = np.argsort(c, kind="stable")
    c_sorted = c[perm]
    counts = np.bincount(c_sorted, minlength=C)
    offs = np.concatenate([[0], np.cumsum(counts)])
    pieces = []
    for cid in range(C):
        pos, en = int(offs[cid]), int(offs[cid + 1])
        while pos < en:
            nxt = min(en, pos + 512)     # moving-operand cap; PSUM straddle is fine
            pieces.append((cid, pos, nxt - pos))
            pos = nxt
    pieces = tuple(pieces)

    key = pieces
    if key not in _cache:
        _cache[key] = _build(pieces)
    nc = _cache[key]

    s_p, p_p, t_p = s[perm], p[perm], times[perm]

    def bt(x):
        return np.ascontiguousarray(x, dtype=BF16)

    x1_in = bt(np.asarray(S1)[s_p].T)
    x2_in = bt(np.asarray(S2)[s_p].T)
    ts_in = bt(np.asarray(T_S)[t_p].T)
    to_in = bt(np.asarray(T_O)[t_p].T)
    pp1 = np.asarray(P1)[p_p]                       # [B, RD]
    pp2 = np.asarray(P2)[p_p]
    G1 = np.asarray(G1); G2 = np.asarray(G2)
    O1 = np.asarray(O1); O2 = np.asarray(O2)
    bnp = np.stack([g11, b11, g12, b12, g21, b21, g22, b22], axis=1).astype(np.float32)
    bnp = np.ascontiguousarray(bnp)

    in_maps = []
    for k in range(NCORES):
        rs = slice(RS * k, RS * (k + 1))
        vs = slice(ES * k, ES * (k + 1))
        g1k = bt(G1[rs].reshape(RS, 2, 100, ED).transpose(2, 0, 1, 3))
        g2k = bt(G2[:, rs].reshape(C, RS, 2, 100, ED).transpose(1, 2, 3, 0, 4))
        pb1 = bt(pp1[:, rs].T)
        pb2 = bt(pp2[:, rs].T)
        o1t = bt(O1[vs].T)   # [200, ES]
        o2t = bt(O2[vs].T)
        in_maps.append({
            "x1_in": x1_in, "x2_in": x2_in, "ts_in": ts_in, "to_in": to_in,
            "g1_in": g1k, "g2_in": g2k, "pb1_in": pb1, "pb2_in": pb2,
            "oc0_in": np.ascontiguousarray(o1t[0:128]),
            "oc1_in": np.ascontiguousarray(o1t[128:200]),
            "oc2_in": np.ascontiguousarray(o2t[0:128]),
            "oc3_in": np.ascontiguousarray(o2t[128:200]),
            "bnp_in": bnp,
        })

    res = run_bass_kernel_spmd(nc, in_maps, core_ids=list(range(NCORES)))

    out_sorted = np.concatenate(
        [res.results[k]["out"].astype(np.float32) for k in range(NCORES)], axis=1)
    out = np.empty_like(out_sorted)
    out[perm] = out_sorted
    return out
